# revision 3
# baseline (speedup 1.0000x reference)
"""Trainium2 Bass kernel for nn_DiffusionDepthController.

Data-parallel over batch B=8 on 8 NeuronCores, one batch per core.

Speed strategy vs the fp32 baseline: every large GEMM runs as fp16 hi/lo
split-matmuls at 1 cyc/row instead of fp32's 4 cyc/row.
- Router GEMM1 (X part): X^T split to (Xh, Xl) fp16 on device, W1x split
  host-side; 3 passes Xh@Wh + Xl@Wh + Xh@Wl -> error ~2^-22 (exact enough
  that the top-2 expert selection matches fp32 bit-for-bit on this input).
- Router LN folded into GEMM as in the baseline; the rank-3 correction
  (zW - mu*s1 + irs*c1) becomes an 8-row fp16 hi/lo product pair.
- GEMM2: hidden gelu output split hi/lo fp16, Wr2 split host-side, 3 passes.
- Conditioning chain: t_embed depends only on step_idx -> computed on host
  and folded into the Wc1 bias.  Remaining chain GEMMs stream host-split
  fp16 hi/lo weights (3-pass with split activations).
Pool + stats + softmax stay fp32.
"""
import os, sys, math
from contextlib import ExitStack
sys.path.insert(0, '/opt/trn_rl_repo')
import numpy as np
import concourse.bacc as bacc
import concourse.mybir as mybir
from concourse import tile
from concourse.masks import make_identity

B, S, D, DC, H, L = 8, 4096, 1024, 1024, 16, 2
TOPK, MAX_DEPTH, EPS = 2, 32, 1e-5
P = 128
NT = S // P          # 32 token tiles
GRP = 512            # tokens per router group
NG = S // GRP        # 8 groups
TPG = GRP // P       # 4 token tiles per group
F32 = mybir.dt.float32
F16 = mybir.dt.float16
AF = mybir.ActivationFunctionType
ALU = mybir.AluOpType
AX = mybir.AxisListType

G1_PASSES = int(os.environ.get("G1_PASSES", "3"))

_CACHE = {}


def _chain_weights():
    # (name, K, N, gelu_after)
    ws = [("W_inp", D, DC, False), ("Wc1x", DC, DC, True), ("Wc2", DC, DC, False)]
    for l in range(L):
        ws += [(f"Wmod{l}", DC, 4 * DC, False), (f"Wm1_{l}", DC, 4 * DC, True),
               (f"Wm2_{l}", 4 * DC, DC, False)]
    ws += [("Wf", DC, DC, False)]
    return ws


def _build(g1_passes=3):
    nc = bacc.Bacc(None, target_bir_lowering=False)
    nc.num_devices = 8

    # ---------------- DRAM I/O ----------------
    Xd = nc.dram_tensor("X", [S, D], F32, kind="ExternalInput")
    z_cm = nc.dram_tensor("z_cm", [P, DC // P], F32, kind="ExternalInput")
    gp_cm = nc.dram_tensor("gp_cm", [P, D // P], F32, kind="ExternalInput")
    bp_cm = nc.dram_tensor("bp_cm", [P, D // P], F32, kind="ExternalInput")
    gz_cm = nc.dram_tensor("gz_cm", [P, DC // P], F32, kind="ExternalInput")
    bz_cm = nc.dram_tensor("bz_cm", [P, DC // P], F32, kind="ExternalInput")
    wdr_h, wdr_l = {}, {}
    bias_cm = {}
    for name, K, N, _ in _chain_weights():
        wdr_h[name] = nc.dram_tensor(name + "_h", [K, N], F16, kind="ExternalInput")
        wdr_l[name] = nc.dram_tensor(name + "_l", [K, N], F16, kind="ExternalInput")
        bias_cm[name] = nc.dram_tensor("b_" + name, [P, N // P], F32, kind="ExternalInput")
    W1xh_d = nc.dram_tensor("W1xh", [D, DC], F16, kind="ExternalInput")
    W1xl_d = nc.dram_tensor("W1xl", [D, DC], F16, kind="ExternalInput")
    W1zh_d = nc.dram_tensor("W1zh", [DC, DC], F16, kind="ExternalInput")
    W1zl_d = nc.dram_tensor("W1zl", [DC, DC], F16, kind="ExternalInput")
    corr_sc_h = nc.dram_tensor("corr_sc_h", [2, DC], F16, kind="ExternalInput")
    corr_sc_l = nc.dram_tensor("corr_sc_l", [2, DC], F16, kind="ExternalInput")
    Wr2h_d = nc.dram_tensor("Wr2h", [DC, H], F16, kind="ExternalInput")
    Wr2l_d = nc.dram_tensor("Wr2l", [DC, H], F16, kind="ExternalInput")
    br2d = nc.dram_tensor("br2", [1, H], F32, kind="ExternalInput")
    ones_sd = nc.dram_tensor("ones_s", [1, S], F32, kind="ExternalInput")
    alphad = nc.dram_tensor("alpha", [S, H], F32, kind="ExternalOutput")

    with tile.TileContext(nc) as tc, ExitStack() as stack:
        const = stack.enter_context(tc.tile_pool(name="const", bufs=1))
        dramp = stack.enter_context(tc.tile_pool(name="dramp", bufs=2, space="DRAM"))

        ident = const.tile([P, P], F32)
        make_identity(nc, ident)
        ones_col = const.tile([P, 1], F32)
        nc.vector.memset(ones_col[:], 1.0)
        ones_row = const.tile([1, 512], F32)
        nc.vector.memset(ones_row[:], 1.0)
        eps_col = const.tile([P, 1], F32)
        nc.vector.memset(eps_col[:], EPS)

        # persistent per-token stats [128, NT]
        rsum_t = const.tile([P, NT], F32)
        rssq_t = const.tile([P, NT], F32)
        rs_rsmu = const.tile([P, NT, 2], F32)

        # =========== PASS 1: stream X, stats + weighted pooling ===========
        with tc.tile_pool(name="p1x", bufs=2) as p1x, \
             tc.tile_pool(name="p1s", bufs=3) as p1s, \
             tc.tile_pool(name="p1ps", bufs=1, space="PSUM") as p1ps:
            gpsum0 = p1ps.tile([1, 512], F32)
            gpsum1 = p1ps.tile([1, 512], F32, name="gpsum1")
            scps = p1ps.tile([1, 2], F32, name="scps")
            CH = 8
            for c in range(NT // CH):
                xs = []
                for j in range(CH):
                    i = c * CH + j
                    x = p1x.tile([P, D], F32, name=f"x{i}", tag=f"x{j}")
                    nc.sync.dma_start(x[:], Xd[i * P:(i + 1) * P, :])
                    sq = p1x.tile([P, D], F32, name=f"sq{i}", tag="sq")
                    nc.scalar.activation(sq[:], x[:], AF.Square,
                                         accum_out=rssq_t[:, i:i + 1])
                    nc.vector.tensor_reduce(rsum_t[:, i:i + 1], x[:], axis=AX.X,
                                            op=ALU.add)
                    xs.append(x)
                sl = slice(c * CH, (c + 1) * CH)
                mu8 = p1s.tile([P, CH], F32, name=f"mu8_{c}", tag="s1")
                nc.vector.tensor_scalar_mul(mu8[:], rsum_t[:, sl], 1.0 / D)
                musq8 = p1s.tile([P, CH], F32, name=f"musq8_{c}", tag="s2")
                nc.vector.tensor_tensor(musq8[:], mu8[:], mu8[:], op=ALU.mult)
                varp8 = p1s.tile([P, CH], F32, name=f"varp8_{c}", tag="s3")
                nc.vector.tensor_scalar(varp8[:], rssq_t[:, sl], 1.0 / D, None,
                                        op0=ALU.mult)
                nc.vector.tensor_tensor(varp8[:], varp8[:], musq8[:], op=ALU.subtract)
                sv8 = p1s.tile([P, CH], F32, name=f"sv8_{c}", tag="s4")
                nc.scalar.activation(sv8[:], varp8[:], AF.Sqrt, bias=eps_col[:])
                nc.vector.reciprocal(rs_rsmu[:, sl, 0], sv8[:])
                nc.vector.tensor_tensor(rs_rsmu[:, sl, 1], rs_rsmu[:, sl, 0], mu8[:],
                                        op=ALU.mult)
                for j in range(CH):
                    i = c * CH + j
                    x = xs[j]
                    nc.tensor.matmul(gpsum0[:], rs_rsmu[:, i, 0:1], x[:, 0:512],
                                     start=(i == 0), stop=(i == NT - 1))
                    nc.tensor.matmul(gpsum1[:], rs_rsmu[:, i, 0:1], x[:, 512:1024],
                                     start=(i == 0), stop=(i == NT - 1))
                    nc.tensor.matmul(scps[:], ones_col[:], rs_rsmu[:, i, :],
                                     start=(i == 0), stop=(i == NT - 1))

            g_row = const.tile([1, D], F32)
            nc.scalar.copy(g_row[:, 0:512], gpsum0[:])
            nc.scalar.copy(g_row[:, 512:1024], gpsum1[:])
            sc_row = const.tile([1, 2], F32)
            nc.scalar.copy(sc_row[:], scps[:])

        # broadcast [sum rs, sum rs*mu] to all partitions
        with tc.tile_pool(name="bcps", bufs=1, space="PSUM") as bcps:
            bps = bcps.tile([P, 2], F32)
            nc.tensor.matmul(bps[:], ones_row[0:1, 0:P], sc_row[:], start=True, stop=True)
            scb = const.tile([P, 2], F32)
            nc.scalar.copy(scb[:], bps[:])

        # corr tensors (filled by the chain below, consumed by pass 2)
        corr_mov8 = const.tile([8, S], F16)
        corr_lhsT8 = const.tile([8, DC], F16)
        rs_row = const.tile([1, S], F32)

        # ---------- helpers ----------
        def bounce_to_cm(row_ap, n, name):
            """SBUF row [1, n] -> DRAM -> SBUF column-major [128, n/128]."""
            scr = dramp.tile([n], F32, name=name + "_scr", tag=f"scr{n}")
            nc.sync.dma_start(scr[None, :], row_ap)
            cmv = scr.rearrange("(c p) -> p c", p=P)
            dst = chn.tile([P, n // P], F32, name=name, tag=f"cm{n}")
            nc.sync.dma_start(dst[:], cmv)
            return dst

        chain_list = _chain_weights()

        # =========== CHAIN (column-major activations, fp16 h/l weights) =====
        with tc.tile_pool(name="chn", bufs=2) as chn:
            with tc.tile_pool(name="wstr", bufs=8) as wstr, \
                 tc.tile_pool(name="crow", bufs=2, space="PSUM") as crow, \
                 tc.tile_pool(name="cmisc", bufs=2, space="PSUM") as cmisc:

                def split16(act, k, nametag):
                    ah = chn.tile([P, k], F16, name=nametag + "_ah", tag="ah",
                                  padded_shape=[P, 32])
                    nc.scalar.copy(ah[:], act[:, 0:k])
                    al = chn.tile([P, k], F16, name=nametag + "_al", tag="al",
                                  padded_shape=[P, 32])
                    nc.vector.tensor_tensor(al[:], act[:, 0:k], ah[:], op=ALU.subtract)
                    return ah, al

                def gemm16_cm(act, K, N, wname, gelu):
                    """act: [128, K/128] f32 col-major -> [128, N/128] f32 col-major."""
                    k = K // P
                    ah, al = split16(act, k, wname)
                    rowbuf = chn.tile([1, N], F32, name=wname + "_row", tag="rowbuf",
                                      padded_shape=[1, 4 * DC])
                    nch = (N + 511) // 512
                    for c in range(nch):
                        n0, n1 = c * 512, min(N, (c + 1) * 512)
                        pr = crow.tile([1, 512], F32, name=wname + f"_ps{c}", tag="prow")
                        for kt in range(k):
                            wh = wstr.tile([P, n1 - n0], F16, name=f"{wname}_wh{c}_{kt}",
                                           tag="wh", padded_shape=[P, 512])
                            nc.sync.dma_start(wh[:], wdr_h[wname][kt * P:(kt + 1) * P, n0:n1])
                            wl = wstr.tile([P, n1 - n0], F16, name=f"{wname}_wl{c}_{kt}",
                                           tag="wl", padded_shape=[P, 512])
                            nc.sync.dma_start(wl[:], wdr_l[wname][kt * P:(kt + 1) * P, n0:n1])
                            nc.tensor.matmul(pr[0:1, 0:n1 - n0], ah[:, kt:kt + 1], wh[:],
                                             start=(kt == 0), stop=False)
                            nc.tensor.matmul(pr[0:1, 0:n1 - n0], al[:, kt:kt + 1], wh[:],
                                             start=False, stop=False)
                            nc.tensor.matmul(pr[0:1, 0:n1 - n0], ah[:, kt:kt + 1], wl[:],
                                             start=False, stop=(kt == k - 1))
                        nc.scalar.copy(rowbuf[:, n0:n1], pr[0:1, 0:n1 - n0])
                    out = bounce_to_cm(rowbuf[:], N, wname + "_cm")
                    bt = chn.tile([P, N // P], F32, name=wname + "_b", tag=f"cm{N}b")
                    nc.sync.dma_start(bt[:], bias_cm[wname][:])
                    out2 = chn.tile([P, N // P], F32, name=wname + "_o", tag=f"cm{N}o")
                    nc.vector.tensor_tensor(out2[:], out[:], bt[:], op=ALU.add)
                    if gelu:
                        nc.scalar.activation(out2[:], out2[:], AF.Gelu)
                    return out2

                def ln_stats_cm(act, nfeat):
                    """col-major [128, k] -> (mu, rs) broadcast [P, 2]."""
                    k = nfeat // P
                    ps = cmisc.tile([1, k], F32, name="lnps", tag="lnps", padded_shape=[1, 32])
                    nc.tensor.matmul(ps[:], ones_col[:], act[:, 0:k], start=True, stop=True)
                    srow = chn.tile([1, 1], F32, name="ssum", tag="s11a")
                    nc.vector.tensor_reduce(srow[:], ps[:], axis=AX.X, op=ALU.add)
                    sqt = chn.tile([P, k], F32, name="sqt", tag="sqt", padded_shape=[P, 32])
                    nc.scalar.activation(sqt[:], act[:, 0:k], AF.Square)
                    ps2 = cmisc.tile([1, k], F32, name="lnps2", tag="lnps2", padded_shape=[1, 32])
                    nc.tensor.matmul(ps2[:], ones_col[:], sqt[:], start=True, stop=True)
                    ssq = chn.tile([1, 1], F32, name="ssq", tag="s11b")
                    nc.vector.tensor_reduce(ssq[:], ps2[:], axis=AX.X, op=ALU.add)
                    mu = chn.tile([1, 1], F32, name="lmu", tag="s11c")
                    nc.scalar.mul(mu[:], srow[:], 1.0 / nfeat)
                    msq = chn.tile([1, 1], F32, name="lmsq", tag="s11d")
                    nc.vector.tensor_tensor(msq[:], mu[:], mu[:], op=ALU.mult)
                    var = chn.tile([1, 1], F32, name="lvar", tag="s11e")
                    nc.vector.tensor_scalar(var[:], ssq[:], 1.0 / nfeat, msq[:],
                                            op0=ALU.mult, op1=ALU.subtract)
                    sv = chn.tile([1, 1], F32, name="lsv", tag="s11f")
                    nc.scalar.activation(sv[:], var[:], AF.Sqrt, bias=eps_col[0:1, :])
                    rs = chn.tile([1, 1], F32, name="lrs", tag="s11g")
                    nc.vector.reciprocal(rs[:], sv[:])
                    murs = chn.tile([1, 2], F32, name="lmurs", tag="s12")
                    nc.scalar.copy(murs[:, 0:1], mu[:])
                    nc.scalar.copy(murs[:, 1:2], rs[:])
                    pb = cmisc.tile([P, 2], F32, name="lnbc", tag="lnbc")
                    nc.tensor.matmul(pb[:], ones_row[0:1, 0:P], murs[:], start=True, stop=True)
                    mb = chn.tile([P, 2], F32, name="lmb", tag="s13")
                    nc.scalar.copy(mb[:], pb[:])
                    return mb

                # g finalize: g_cm = gp/S*(pool - sum(rs*mu)) + bp   (col-major)
                g0 = bounce_to_cm(g_row[:], D, "g0")
                gpc = chn.tile([P, D // P], F32, name="gpc", tag="cmgp")
                nc.sync.dma_start(gpc[:], gp_cm[:])
                bpc = chn.tile([P, D // P], F32, name="bpc", tag="cmbp")
                nc.sync.dma_start(bpc[:], bp_cm[:])
                g1t = chn.tile([P, D // P], F32, name="g1t", tag="cmg1")
                nc.vector.tensor_scalar(g1t[:], g0[:], scb[:, 1:2], 1.0 / S,
                                        op0=ALU.subtract, op1=ALU.mult)
                g2t = chn.tile([P, D // P], F32, name="g2t", tag="cmg2")
                nc.vector.tensor_tensor(g2t[:], g1t[:], gpc[:], op=ALU.mult)
                g_cm = chn.tile([P, D // P], F32, name="g_cm", tag="cmg3")
                nc.vector.tensor_tensor(g_cm[:], g2t[:], bpc[:], op=ALU.add)

                # chain (t_embed folded into Wc1x bias host-side)
                x_ctrl = gemm16_cm(g_cm, D, DC, "W_inp", False)
                c1t_ = gemm16_cm(x_ctrl, DC, DC, "Wc1x", True)
                cond = gemm16_cm(c1t_, DC, DC, "Wc2", False)
                gcond = chn.tile([P, DC // P], F32, name="gcond", tag="cmgc")
                nc.scalar.activation(gcond[:], cond[:], AF.Gelu)

                zc = chn.tile([P, DC // P], F32, name="zc0", tag="cmz0")
                nc.sync.dma_start(zc[:], z_cm[:])
                z_cur = zc
                for l in range(L):
                    mod = gemm16_cm(gcond, DC, 4 * DC, f"Wmod{l}", False)  # [128, 32]
                    kk = DC // P
                    s1a, sh1a = mod[:, 0:kk], mod[:, kk:2 * kk]
                    s2a, sh2a = mod[:, 2 * kk:3 * kk], mod[:, 3 * kk:4 * kk]
                    mb = ln_stats_cm(z_cur, DC)
                    lnz = chn.tile([P, kk], F32, name=f"lnz{l}", tag="cmlnz")
                    nc.vector.tensor_scalar(lnz[:], z_cur[:], mb[:, 0:1], mb[:, 1:2],
                                            op0=ALU.subtract, op1=ALU.mult)
                    s1p = chn.tile([P, kk], F32, name=f"s1p{l}", tag="cms1p")
                    nc.scalar.add(s1p[:], s1a, 1.0)
                    h0 = chn.tile([P, kk], F32, name=f"h0_{l}", tag="cmh0")
                    nc.vector.tensor_tensor(h0[:], lnz[:], s1p[:], op=ALU.mult)
                    h1 = chn.tile([P, kk], F32, name=f"h1_{l}", tag="cmh1")
                    nc.vector.tensor_tensor(h1[:], h0[:], sh1a, op=ALU.add)
                    h2 = gemm16_cm(h1, DC, 4 * DC, f"Wm1_{l}", True)
                    h3 = gemm16_cm(h2, 4 * DC, DC, f"Wm2_{l}", False)
                    s2p = chn.tile([P, kk], F32, name=f"s2p{l}", tag="cms2p")
                    nc.scalar.add(s2p[:], s2a, 1.0)
                    h4 = chn.tile([P, kk], F32, name=f"h4_{l}", tag="cmh4")
                    nc.vector.tensor_tensor(h4[:], h3[:], s2p[:], op=ALU.mult)
                    h5 = chn.tile([P, kk], F32, name=f"h5_{l}", tag="cmh5")
                    nc.vector.tensor_tensor(h5[:], h4[:], sh2a, op=ALU.add)
                    zn = chn.tile([P, kk], F32, name=f"zn{l}", tag=f"cmzn{l % 2}")
                    nc.vector.tensor_tensor(zn[:], z_cur[:], h5[:], op=ALU.add)
                    z_cur = zn

                zf = gemm16_cm(z_cur, DC, DC, "Wf", False)
                mb = ln_stats_cm(zf, DC)
                zno = chn.tile([P, DC // P], F32, name="zno", tag="cmzno")
                nc.vector.tensor_scalar(zno[:], zf[:], mb[:, 0:1], mb[:, 1:2],
                                        op0=ALU.subtract, op1=ALU.mult)
                gzc = chn.tile([P, DC // P], F32, name="gzc", tag="cmgz")
                nc.sync.dma_start(gzc[:], gz_cm[:])
                bzc = chn.tile([P, DC // P], F32, name="bzc", tag="cmbz")
                nc.sync.dma_start(bzc[:], bz_cm[:])
                zf2 = chn.tile([P, DC // P], F32, name="zf2", tag="cmzf2")
                nc.vector.tensor_tensor(zf2[:], zno[:], gzc[:], op=ALU.mult)
                z_fin = chn.tile([P, DC // P], F32, name="z_fin", tag="cmzf3")
                nc.vector.tensor_tensor(z_fin[:], zf2[:], bzc[:], op=ALU.add)

                # ---- zW row = z_fin @ W1z  (fp16 3-pass, row-major out) ----
                zfh, zfl = split16(z_fin, DC // P, "zf")
                zw_row = chn.tile([1, DC], F32, name="zw_row", tag="rowzw")
                for c in range(2):
                    pr = crow.tile([1, 512], F32, name=f"zwps{c}", tag="prow")
                    for kt in range(DC // P):
                        wzh = wstr.tile([P, 512], F16, name=f"w1zh_{c}_{kt}", tag="wh")
                        nc.sync.dma_start(wzh[:], W1zh_d[kt * P:(kt + 1) * P, c * 512:(c + 1) * 512])
                        wzl = wstr.tile([P, 512], F16, name=f"w1zl_{c}_{kt}", tag="wl")
                        nc.sync.dma_start(wzl[:], W1zl_d[kt * P:(kt + 1) * P, c * 512:(c + 1) * 512])
                        nc.tensor.matmul(pr[:], zfh[:, kt:kt + 1], wzh[:],
                                         start=(kt == 0), stop=False)
                        nc.tensor.matmul(pr[:], zfl[:, kt:kt + 1], wzh[:],
                                         start=False, stop=False)
                        nc.tensor.matmul(pr[:], zfh[:, kt:kt + 1], wzl[:],
                                         start=False, stop=(kt == DC // P - 1))
                    nc.scalar.copy(zw_row[:, c * 512:(c + 1) * 512], pr[:])

                # ---- z scalar stats for router LN ----
                zsq = chn.tile([P, DC // P], F32, name="zsq", tag="cmzsq")
                nc.scalar.activation(zsq[:], z_fin[:], AF.Square)
                psa = cmisc.tile([1, DC // P], F32, name="zsps", tag="lnps", padded_shape=[1, 32])
                nc.tensor.matmul(psa[:], ones_col[:], z_fin[:], start=True, stop=True)
                psb = cmisc.tile([1, DC // P], F32, name="zsps2", tag="lnps2", padded_shape=[1, 32])
                nc.tensor.matmul(psb[:], ones_col[:], zsq[:], start=True, stop=True)
                zsr = chn.tile([1, 2], F32, name="zsr", tag="s12b")
                nc.vector.tensor_reduce(zsr[:, 0:1], psa[:], axis=AX.X, op=ALU.add)
                nc.vector.tensor_reduce(zsr[:, 1:2], psb[:], axis=AX.X, op=ALU.add)
                pzb = cmisc.tile([P, 2], F32, name="pzb", tag="lnbc")
                nc.tensor.matmul(pzb[:], ones_row[0:1, 0:P], zsr[:], start=True, stop=True)
                zsb = const.tile([P, 2], F32)
                nc.scalar.copy(zsb[:], pzb[:])

                # ---- router per-token stats [128, NT] ----
                DD = D + DC
                mur = const.tile([P, NT], F32)
                nc.vector.tensor_scalar(mur[:], rsum_t[:].broadcast_to([P, NT]), zsb[:, 0:1],
                                        1.0 / DD, op0=ALU.add, op1=ALU.mult)
                mq2 = chn.tile([P, NT], F32, name="mq2", tag="st1")
                nc.vector.tensor_tensor(mq2[:], mur[:], mur[:], op=ALU.mult)
                vr = chn.tile([P, NT], F32, name="vr", tag="st2")
                nc.vector.tensor_scalar(vr[:], rssq_t[:], zsb[:, 1:2], 1.0 / DD,
                                        op0=ALU.add, op1=ALU.mult)
                vr2 = chn.tile([P, NT], F32, name="vr2", tag="st3")
                nc.vector.tensor_tensor(vr2[:], vr[:], mq2[:], op=ALU.subtract)
                irs = const.tile([P, NT], F32)  # 1/rs = sqrt(var+eps)
                nc.scalar.activation(irs[:], vr2[:], AF.Sqrt, bias=eps_col[:])
                rst = const.tile([P, NT], F32)
                nc.vector.reciprocal(rst[:], irs[:])

                # fp16 h/l splits of mur, irs
                murh = const.tile([P, NT], F16)
                nc.scalar.copy(murh[:], mur[:])
                murl = const.tile([P, NT], F16)
                nc.vector.tensor_tensor(murl[:], mur[:], murh[:], op=ALU.subtract)
                irsh = const.tile([P, NT], F16)
                nc.scalar.copy(irsh[:], irs[:])
                irsl = const.tile([P, NT], F16)
                nc.vector.tensor_tensor(irsl[:], irs[:], irsh[:], op=ALU.subtract)

                # corr_mov8 rows: [mu_h, irs_h, 1, mu_h, irs_h, 1, mu_l, irs_l]
                ones16 = chn.tile([1, S], F16, name="ones16", tag="ones16")
                nc.vector.memset(ones16[:], 1.0)
                o16scr = dramp.tile([S], F16, name="o16scr", tag="o16scr")
                nc.sync.dma_start(o16scr[None, :], ones16[:])
                nc.sync.dma_start(corr_mov8[2:3, :], o16scr[None, :])
                nc.sync.dma_start(corr_mov8[5:6, :], o16scr[None, :])
                for srcst, rows, tg in ((murh, (0, 3), "a"), (irsh, (1, 4), "b"),
                                        (murl, (6,), "c"), (irsl, (7,), "d")):
                    scr = dramp.tile([P, NT], F16, name=f"st16{tg}", tag="stscr16")
                    nc.sync.dma_start(scr[:], srcst[:])
                    for r in rows:
                        nc.sync.dma_start(
                            corr_mov8[r:r + 1, :].rearrange("o (t p) -> o t p", t=NT),
                            scr.rearrange("p t -> t p"))
                # rs_row fp32 (for the pre-gelu scale)
                scr = dramp.tile([P, NT], F32, name="stscr2", tag="stscr")
                nc.sync.dma_start(scr[:], rst[:])
                nc.sync.dma_start(rs_row[:].rearrange("o (t p) -> o t p", t=NT),
                                  scr.rearrange("p t -> t p"))

                # corr_lhsT8 rows: [-s1_h, c1_h, zW_h, -s1_l, c1_l, zW_l, -s1_h, c1_h]
                nc.sync.dma_start(corr_lhsT8[0:1, :], corr_sc_h[0:1, :])
                nc.sync.dma_start(corr_lhsT8[6:7, :], corr_sc_h[0:1, :])
                nc.sync.dma_start(corr_lhsT8[1:2, :], corr_sc_h[1:2, :])
                nc.sync.dma_start(corr_lhsT8[7:8, :], corr_sc_h[1:2, :])
                nc.sync.dma_start(corr_lhsT8[3:4, :], corr_sc_l[0:1, :])
                nc.sync.dma_start(corr_lhsT8[4:5, :], corr_sc_l[1:2, :])
                zwh = chn.tile([1, DC], F16, name="zwh", tag="rowzwh")
                nc.scalar.copy(zwh[:], zw_row[:])
                zwl = chn.tile([1, DC], F16, name="zwl", tag="rowzwl")
                nc.vector.tensor_tensor(zwl[:], zw_row[:], zwh[:], op=ALU.subtract)
                zwscrh = dramp.tile([DC], F16, name="zwscrh", tag="zwscr16h")
                nc.sync.dma_start(zwscrh[None, :], zwh[:])
                nc.sync.dma_start(corr_lhsT8[2:3, :], zwscrh[None, :])
                zwscrl = dramp.tile([DC], F16, name="zwscrl", tag="zwscr16l")
                nc.sync.dma_start(zwscrl[None, :], zwl[:])
                nc.sync.dma_start(corr_lhsT8[5:6, :], zwscrl[None, :])

        # =========== PASS 2: router (fp16 split GEMMs) ===========
        w1xh_sb = const.tile([P, D // P, DC], F16)
        w1xl_sb = const.tile([P, D // P, DC], F16)
        for kt in range(D // P):
            nc.sync.dma_start(w1xh_sb[:, kt, :], W1xh_d[kt * P:(kt + 1) * P, :])
            if g1_passes == 3:
                nc.sync.dma_start(w1xl_sb[:, kt, :], W1xl_d[kt * P:(kt + 1) * P, :])
        wr2h_sb = const.tile([P, DC // P, H], F16)
        wr2l_sb = const.tile([P, DC // P, H], F16)
        for kt in range(DC // P):
            nc.sync.dma_start(wr2h_sb[:, kt, :], Wr2h_d[kt * P:(kt + 1) * P, :])
            nc.sync.dma_start(wr2l_sb[:, kt, :], Wr2l_d[kt * P:(kt + 1) * P, :])
        br2_sb = const.tile([1, H], F32)
        nc.sync.dma_start(br2_sb[:], br2d[:])

        with tc.tile_pool(name="p2x", bufs=3) as p2x, \
             tc.tile_pool(name="p2xt", bufs=2) as p2xt, \
             tc.tile_pool(name="p2g", bufs=2) as p2g, \
             tc.tile_pool(name="p2s", bufs=3) as p2s, \
             tc.tile_pool(name="pT", bufs=2, space="PSUM") as pT, \
             tc.tile_pool(name="pG", bufs=2, space="PSUM") as pG, \
             tc.tile_pool(name="pM", bufs=1, space="PSUM") as pM:
            for g in range(NG):
                xtgh = p2xt.tile([P, D // P, GRP], F16, name=f"xtgh{g}", tag="xtgh")
                xtgl = p2xt.tile([P, D // P, GRP], F16, name=f"xtgl{g}", tag="xtgl")
                for j in range(TPG):
                    i = g * TPG + j
                    x = p2x.tile([P, D], F32, name=f"x2_{i}", tag="x2")
                    nc.sync.dma_start(x[:], Xd[i * P:(i + 1) * P, :])
                    for bblk in range(D // P):
                        pt = pT.tile([P, P], F32, name=f"pt{i}_{bblk}", tag="pt")
                        nc.tensor.transpose(pt[:], x[:, bblk * P:(bblk + 1) * P], ident[:])
                        nc.scalar.copy(xtgh[:, bblk, j * P:(j + 1) * P], pt[:])
                        nc.vector.tensor_tensor(xtgl[:, bblk, j * P:(j + 1) * P], pt[:],
                                                xtgh[:, bblk, j * P:(j + 1) * P],
                                                op=ALU.subtract)
                # rs broadcast [128, GRP]
                pb = pM.tile([P, GRP], F32, name=f"pb{g}", tag="pbg")
                nc.tensor.matmul(pb[:], ones_row[0:1, 0:P],
                                 rs_row[0:1, g * GRP:(g + 1) * GRP], start=True, stop=True)
                rsb = p2s.tile([P, GRP], F32, name=f"rsb{g}", tag="rsb")
                nc.vector.tensor_copy(rsb[:], pb[:])

                g1h = p2g.tile([P, DC // P, GRP], F16, name=f"g1h_{g}", tag="g1h")
                g1l = p2g.tile([P, DC // P, GRP], F16, name=f"g1l_{g}", tag="g1l")
                for n in range(DC // P):
                    pg = pG.tile([P, GRP], F32, name=f"pg{g}_{n}", tag="pg")
                    for kt in range(D // P):
                        nc.tensor.matmul(pg[:], w1xh_sb[:, kt, n * P:(n + 1) * P],
                                         xtgh[:, kt, :], start=(kt == 0), stop=False)
                        nc.tensor.matmul(pg[:], w1xh_sb[:, kt, n * P:(n + 1) * P],
                                         xtgl[:, kt, :], start=False, stop=False)
                        if g1_passes == 3:
                            nc.tensor.matmul(pg[:], w1xl_sb[:, kt, n * P:(n + 1) * P],
                                             xtgh[:, kt, :], start=False, stop=False)
                    nc.tensor.matmul(pg[:], corr_lhsT8[:, n * P:(n + 1) * P],
                                     corr_mov8[:, g * GRP:(g + 1) * GRP],
                                     start=False, stop=True)
                    pre = p2s.tile([P, GRP], F32, name=f"pre{g}_{n}", tag="pre")
                    nc.vector.tensor_tensor(pre[:], pg[:], rsb[:], op=ALU.mult)
                    g32 = p2s.tile([P, GRP], F32, name=f"g32_{g}_{n}", tag="g32")
                    nc.scalar.activation(g32[:], pre[:], AF.Gelu)
                    nc.vector.tensor_copy(g1h[:, n, :], g32[:])
                    nc.vector.tensor_tensor(g1l[:, n, :], g32[:], g1h[:, n, :],
                                            op=ALU.subtract)

                # GEMM2 -> logits2^T [16, GRP] (fp16 3-pass)
                pl = pM.tile([H, GRP], F32, name=f"pl{g}", tag="pl")
                for kt in range(DC // P):
                    nc.tensor.matmul(pl[:], wr2h_sb[:, kt, :], g1h[:, kt, :],
                                     start=(kt == 0), stop=False)
                    nc.tensor.matmul(pl[:], wr2h_sb[:, kt, :], g1l[:, kt, :],
                                     start=False, stop=False)
                    nc.tensor.matmul(pl[:], wr2l_sb[:, kt, :], g1h[:, kt, :],
                                     start=False, stop=False)
                nc.tensor.matmul(pl[:], br2_sb[:], ones_row[0:1, 0:GRP],
                                 start=False, stop=True)
                l2t = p2s.tile([H, GRP], F32, name=f"l2t{g}", tag="l2t")
                nc.scalar.copy(l2t[:], pl[:])

                for j in range(TPG):
                    ptb = pM.tile([P, H], F32, name=f"ptb{g}_{j}", tag="ptb")
                    nc.tensor.transpose(ptb[:], l2t[:, j * P:(j + 1) * P],
                                        ident[0:H, 0:H])
                    e = p2s.tile([P, H], F32, name=f"e{g}{j}", tag="te")
                    nc.scalar.activation(e[:], ptb[:], AF.Exp)
                    m1 = p2s.tile([P, 1], F32, name=f"m1{g}{j}", tag="tm1")
                    nc.vector.reduce_max(m1[:], e[:], axis=AX.X)
                    mask = p2s.tile([P, H], F32, name=f"mk{g}{j}", tag="tmk")
                    nc.vector.tensor_scalar(mask[:], e[:], m1[:], None, op0=ALU.is_ge)
                    e2 = p2s.tile([P, H], F32, name=f"e2{g}{j}", tag="te2")
                    nc.vector.scalar_tensor_tensor(e2[:], in0=mask[:], scalar=-1e30,
                                                   in1=e[:], op0=ALU.mult, op1=ALU.add)
                    m2 = p2s.tile([P, 1], F32, name=f"m2{g}{j}", tag="tm2")
                    nc.vector.reduce_max(m2[:], e2[:], axis=AX.X)
                    den = p2s.tile([P, 1], F32, name=f"dn{g}{j}", tag="tdn")
                    nc.vector.tensor_tensor(den[:], m1[:], m2[:], op=ALU.add)
                    rden = p2s.tile([P, 1], F32, name=f"rd{g}{j}", tag="trd")
                    nc.vector.reciprocal(rden[:], den[:])
                    keep = p2s.tile([P, H], F32, name=f"kp{g}{j}", tag="tkp")
                    nc.vector.tensor_scalar(keep[:], e[:], m2[:], None, op0=ALU.is_ge)
                    alph = p2s.tile([P, H], F32, name=f"al{g}{j}", tag="tal")
                    nc.vector.scalar_tensor_tensor(alph[:], in0=e[:], scalar=rden[:],
                                                   in1=keep[:], op0=ALU.mult, op1=ALU.mult)
                    i = g * TPG + j
                    nc.sync.dma_start(alphad[i * P:(i + 1) * P, :], alph[:])

    nc.compile()
    return nc


def _cm(v):
    v = np.asarray(v, np.float32).reshape(-1)
    return np.ascontiguousarray(v.reshape(-1, P).T)


def _f16_split(W):
    W = np.asarray(W, np.float64)
    Wh = np.asarray(W, np.float16)
    Wl = np.asarray(W - Wh.astype(np.float64), np.float16)
    return np.ascontiguousarray(Wh), np.ascontiguousarray(Wl)


def _host_prep(inputs):
    X = np.asarray(inputs['X'], np.float32)
    z = np.asarray(inputs['z'], np.float32)
    step_idx = int(inputs['step_idx'])
    t = np.linspace(0.0, 1.0, MAX_DEPTH)
    sig = float(np.clip(np.cos(t * (math.pi / 2)), 1e-4, None)[min(step_idx, MAX_DEPTH - 1)])
    half = DC // 2
    freqs = np.exp(-math.log(10000.0) * np.arange(half, dtype=np.float64) / half)
    args = sig * freqs
    emb = np.concatenate([np.cos(args), np.sin(args)])  # [DC] fp64

    # host-computed t_embed (depends only on step_idx)
    from scipy.special import erf

    def gelu64(x):
        return 0.5 * x * (1 + erf(x / np.sqrt(2.0)))

    Wt1 = np.asarray(inputs['Wt1'], np.float64)
    bt1 = np.asarray(inputs['bt1'], np.float64)
    Wt2 = np.asarray(inputs['Wt2'], np.float64)
    bt2 = np.asarray(inputs['bt2'], np.float64)
    t_embed = gelu64(emb @ Wt1 + bt1) @ Wt2 + bt2  # [DC]

    Wc1 = np.asarray(inputs['Wc1'], np.float64)
    bc1 = np.asarray(inputs['bc1'], np.float64)
    cond_bias = t_embed @ Wc1[DC:] + bc1           # [DC]
    Wc1x = Wc1[:DC]

    gr = np.asarray(inputs['gr'], np.float64)
    br = np.asarray(inputs['br'], np.float64)
    Wr1 = np.asarray(inputs['Wr1'], np.float64)
    W1p = gr[:, None] * Wr1
    W1x = W1p[:D]
    W1z = W1p[D:]
    s1 = W1p.sum(0)
    br1 = np.asarray(inputs['br1'], np.float64)
    c1 = br @ Wr1 + br1

    W1xh, W1xl = _f16_split(W1x)
    W1zh, W1zl = _f16_split(W1z)
    Wr2h, Wr2l = _f16_split(inputs['Wr2'])
    csc = np.stack([-s1, c1])
    csc_h = np.asarray(csc, np.float16)
    csc_l = np.asarray(csc - csc_h.astype(np.float64), np.float16)

    shared = {
        'gp_cm': _cm(inputs['g_pool']), 'bp_cm': _cm(inputs['b_pool']),
        'gz_cm': _cm(inputs['gz']), 'bz_cm': _cm(inputs['bz']),
        'W1xh': W1xh, 'W1xl': W1xl, 'W1zh': W1zh, 'W1zl': W1zl,
        'corr_sc_h': np.ascontiguousarray(csc_h),
        'corr_sc_l': np.ascontiguousarray(csc_l),
        'Wr2h': Wr2h, 'Wr2l': Wr2l,
        'ones_s': np.ones((1, S), np.float32),
        'br2': np.asarray(inputs['br2'], np.float32)[None, :],
    }
    cw = {'W_inp': inputs['W_inp'], 'Wc1x': Wc1x, 'Wc2': inputs['Wc2'],
          'Wf': inputs['Wf']}
    cb = {'W_inp': inputs['b_inp'], 'Wc1x': cond_bias, 'Wc2': inputs['bc2'],
          'Wf': inputs['bf']}
    for l in range(L):
        cw[f'Wmod{l}'] = np.asarray(inputs['W_mod'])[l]
        cb[f'Wmod{l}'] = np.asarray(inputs['b_mod'])[l]
        cw[f'Wm1_{l}'] = np.asarray(inputs['Wm1'])[l]
        cb[f'Wm1_{l}'] = np.asarray(inputs['bm1'])[l]
        cw[f'Wm2_{l}'] = np.asarray(inputs['Wm2'])[l]
        cb[f'Wm2_{l}'] = np.asarray(inputs['bm2'])[l]
    for k, v in cw.items():
        h, lo = _f16_split(v)
        shared[k + '_h'] = h
        shared[k + '_l'] = lo
        shared['b_' + k] = _cm(np.asarray(cb[k], np.float32))

    in_maps = []
    for c in range(B):
        m = dict(shared)
        m['X'] = np.ascontiguousarray(X[c])
        m['z_cm'] = _cm(z[c])
        in_maps.append(m)
    return in_maps


def get_nc():
    key = ('nc', G1_PASSES)
    if key not in _CACHE:
        _CACHE[key] = _build(g1_passes=G1_PASSES)
    return _CACHE[key]


def kernel(**inputs):
    from concourse.bass_utils import run_bass_kernel_spmd
    nc = get_nc()
    in_maps = _host_prep(inputs)
    res = run_bass_kernel_spmd(nc, in_maps, list(range(B)))
    out = np.stack([res.results[c]['alpha'] for c in range(B)], axis=0)
    return out.astype(np.float32)


# revision 6
# speedup vs baseline: 1.2338x; 1.2338x over previous
"""Trainium2 Bass kernel for nn_DiffusionDepthController.

Data-parallel over batch B=8 on 8 NeuronCores, one batch per core.

Speed strategy vs the fp32 baseline: every large GEMM runs as fp16 hi/lo
split-matmuls at 1 cyc/row instead of fp32's 4 cyc/row.
- Router GEMM1 (X part): X^T split to (Xh, Xl) fp16 on device, W1x split
  host-side; 3 passes Xh@Wh + Xl@Wh + Xh@Wl -> error ~2^-22 (exact enough
  that the top-2 expert selection matches fp32 bit-for-bit on this input).
- Router LN folded into GEMM as in the baseline; the rank-3 correction
  (zW - mu*s1 + irs*c1) becomes an 8-row fp16 hi/lo product pair.
- GEMM2: hidden gelu output split hi/lo fp16, Wr2 split host-side, 3 passes.
- Conditioning chain: t_embed depends only on step_idx -> computed on host
  and folded into the Wc1 bias.  Remaining chain GEMMs stream host-split
  fp16 hi/lo weights (3-pass with split activations).
Pool + stats + softmax stay fp32.
"""
import os, sys, math
from contextlib import ExitStack
sys.path.insert(0, '/opt/trn_rl_repo')
import numpy as np
import concourse.bacc as bacc
import concourse.mybir as mybir
from concourse import tile
from concourse.masks import make_identity

B, S, D, DC, H, L = 8, 4096, 1024, 1024, 16, 2
TOPK, MAX_DEPTH, EPS = 2, 32, 1e-5
P = 128
NT = S // P          # 32 token tiles
GRP = 512            # tokens per router group
NG = S // GRP        # 8 groups
TPG = GRP // P       # 4 token tiles per group
F32 = mybir.dt.float32
F16 = mybir.dt.bfloat16   # bf16: the PE's 1 cyc/row fast path (fp16 is not)
AF = mybir.ActivationFunctionType
ALU = mybir.AluOpType
AX = mybir.AxisListType

G1_PASSES = int(os.environ.get("G1_PASSES", "3"))

_CACHE = {}


def _chain_weights():
    # (name, K, N, gelu_after)
    ws = [("W_inp", D, DC, False), ("Wc1x", DC, DC, True), ("Wc2", DC, DC, False)]
    for l in range(L):
        ws += [(f"Wmod{l}", DC, 4 * DC, False), (f"Wm1_{l}", DC, 4 * DC, True),
               (f"Wm2_{l}", 4 * DC, DC, False)]
    ws += [("Wf", DC, DC, False)]
    return ws


def _build(g1_passes=3):
    nc = bacc.Bacc(None, target_bir_lowering=False)
    nc.num_devices = 8

    # ---------------- DRAM I/O ----------------
    Xd = nc.dram_tensor("X", [S, D], F32, kind="ExternalInput")
    z_cm = nc.dram_tensor("z_cm", [P, DC // P], F32, kind="ExternalInput")
    gp_cm = nc.dram_tensor("gp_cm", [P, D // P], F32, kind="ExternalInput")
    bp_cm = nc.dram_tensor("bp_cm", [P, D // P], F32, kind="ExternalInput")
    gz_cm = nc.dram_tensor("gz_cm", [P, DC // P], F32, kind="ExternalInput")
    bz_cm = nc.dram_tensor("bz_cm", [P, DC // P], F32, kind="ExternalInput")
    wdr_h, wdr_l = {}, {}
    bias_cm = {}
    for name, K, N, _ in _chain_weights():
        wdr_h[name] = nc.dram_tensor(name + "_h", [K, N], F16, kind="ExternalInput")
        wdr_l[name] = nc.dram_tensor(name + "_l", [K, N], F16, kind="ExternalInput")
        bias_cm[name] = nc.dram_tensor("b_" + name, [P, N // P], F32, kind="ExternalInput")
    W1xh_d = nc.dram_tensor("W1xh", [D, DC], F16, kind="ExternalInput")
    W1xl_d = nc.dram_tensor("W1xl", [D, DC], F16, kind="ExternalInput")
    W1zh_d = nc.dram_tensor("W1zh", [DC, DC], F16, kind="ExternalInput")
    W1zl_d = nc.dram_tensor("W1zl", [DC, DC], F16, kind="ExternalInput")
    corr_sc_h = nc.dram_tensor("corr_sc_h", [2, DC], F16, kind="ExternalInput")
    corr_sc_l = nc.dram_tensor("corr_sc_l", [2, DC], F16, kind="ExternalInput")
    Wr2h_d = nc.dram_tensor("Wr2h", [DC, H], F16, kind="ExternalInput")
    Wr2l_d = nc.dram_tensor("Wr2l", [DC, H], F16, kind="ExternalInput")
    br2d = nc.dram_tensor("br2", [1, H], F32, kind="ExternalInput")
    ones_sd = nc.dram_tensor("ones_s", [1, S], F32, kind="ExternalInput")
    alphad = nc.dram_tensor("alpha", [S, H], F32, kind="ExternalOutput")

    with tile.TileContext(nc) as tc, ExitStack() as stack:
        const = stack.enter_context(tc.tile_pool(name="const", bufs=1))
        dramp = stack.enter_context(tc.tile_pool(name="dramp", bufs=2, space="DRAM"))

        ident = const.tile([P, P], F32)
        make_identity(nc, ident)
        ones_col = const.tile([P, 1], F32)
        nc.vector.memset(ones_col[:], 1.0)
        ones_row = const.tile([1, 512], F32)
        nc.vector.memset(ones_row[:], 1.0)
        eps_col = const.tile([P, 1], F32)
        nc.vector.memset(eps_col[:], EPS)

        # persistent per-token stats [128, NT]
        rsum_t = const.tile([P, NT], F32)
        rssq_t = const.tile([P, NT], F32)
        rs_rsmu = const.tile([P, NT, 2], F32)

        # =========== PASS 1: stream X, stats + weighted pooling ===========
        with tc.tile_pool(name="p1x", bufs=2) as p1x, \
             tc.tile_pool(name="p1s", bufs=3) as p1s, \
             tc.tile_pool(name="p1ps", bufs=1, space="PSUM") as p1ps:
            gpsum0 = p1ps.tile([1, 512], F32)
            gpsum1 = p1ps.tile([1, 512], F32, name="gpsum1")
            scps = p1ps.tile([1, 2], F32, name="scps")
            CH = 8
            for c in range(NT // CH):
                xs = []
                for j in range(CH):
                    i = c * CH + j
                    x = p1x.tile([P, D], F32, name=f"x{i}", tag=f"x{j}")
                    nc.sync.dma_start(x[:], Xd[i * P:(i + 1) * P, :])
                    sq = p1x.tile([P, D], F32, name=f"sq{i}", tag="sq")
                    nc.scalar.activation(sq[:], x[:], AF.Square,
                                         accum_out=rssq_t[:, i:i + 1])
                    nc.vector.tensor_reduce(rsum_t[:, i:i + 1], x[:], axis=AX.X,
                                            op=ALU.add)
                    xs.append(x)
                sl = slice(c * CH, (c + 1) * CH)
                mu8 = p1s.tile([P, CH], F32, name=f"mu8_{c}", tag="s1")
                nc.vector.tensor_scalar_mul(mu8[:], rsum_t[:, sl], 1.0 / D)
                musq8 = p1s.tile([P, CH], F32, name=f"musq8_{c}", tag="s2")
                nc.vector.tensor_tensor(musq8[:], mu8[:], mu8[:], op=ALU.mult)
                varp8 = p1s.tile([P, CH], F32, name=f"varp8_{c}", tag="s3")
                nc.vector.tensor_scalar(varp8[:], rssq_t[:, sl], 1.0 / D, None,
                                        op0=ALU.mult)
                nc.vector.tensor_tensor(varp8[:], varp8[:], musq8[:], op=ALU.subtract)
                sv8 = p1s.tile([P, CH], F32, name=f"sv8_{c}", tag="s4")
                nc.scalar.activation(sv8[:], varp8[:], AF.Sqrt, bias=eps_col[:])
                nc.vector.reciprocal(rs_rsmu[:, sl, 0], sv8[:])
                nc.vector.tensor_tensor(rs_rsmu[:, sl, 1], rs_rsmu[:, sl, 0], mu8[:],
                                        op=ALU.mult)
                for j in range(CH):
                    i = c * CH + j
                    x = xs[j]
                    nc.tensor.matmul(gpsum0[:], rs_rsmu[:, i, 0:1], x[:, 0:512],
                                     start=(i == 0), stop=(i == NT - 1))
                    nc.tensor.matmul(gpsum1[:], rs_rsmu[:, i, 0:1], x[:, 512:1024],
                                     start=(i == 0), stop=(i == NT - 1))
                    nc.tensor.matmul(scps[:], ones_col[:], rs_rsmu[:, i, :],
                                     start=(i == 0), stop=(i == NT - 1))

            g_row = const.tile([1, D], F32)
            nc.scalar.copy(g_row[:, 0:512], gpsum0[:])
            nc.scalar.copy(g_row[:, 512:1024], gpsum1[:])
            sc_row = const.tile([1, 2], F32)
            nc.scalar.copy(sc_row[:], scps[:])

        # broadcast [sum rs, sum rs*mu] to all partitions
        with tc.tile_pool(name="bcps", bufs=1, space="PSUM") as bcps:
            bps = bcps.tile([P, 2], F32)
            nc.tensor.matmul(bps[:], ones_row[0:1, 0:P], sc_row[:], start=True, stop=True)
            scb = const.tile([P, 2], F32)
            nc.scalar.copy(scb[:], bps[:])

        # corr tensors (filled by the chain below, consumed by pass 2)
        corr_mov8 = const.tile([8, S], F16)
        corr_lhsT8 = const.tile([8, DC], F16)
        rs_row = const.tile([1, S], F32)

        # ---------- helpers ----------
        def bounce_to_cm(row_ap, n, name):
            """SBUF row [1, n] -> DRAM -> SBUF column-major [128, n/128]."""
            scr = dramp.tile([n], F32, name=name + "_scr", tag=f"scr{n}")
            nc.sync.dma_start(scr[None, :], row_ap)
            cmv = scr.rearrange("(c p) -> p c", p=P)
            dst = chn.tile([P, n // P], F32, name=name, tag=f"cm{n}")
            nc.sync.dma_start(dst[:], cmv)
            return dst

        chain_list = _chain_weights()

        # =========== CHAIN (column-major activations, fp16 h/l weights) =====
        with tc.tile_pool(name="chn", bufs=2) as chn:
            with tc.tile_pool(name="wstr", bufs=8) as wstr, \
                 tc.tile_pool(name="crow", bufs=2, space="PSUM") as crow, \
                 tc.tile_pool(name="cmisc", bufs=2, space="PSUM") as cmisc:

                def split16(act, k, nametag):
                    ah = chn.tile([P, k], F16, name=nametag + "_ah", tag="ah",
                                  padded_shape=[P, 32])
                    nc.scalar.copy(ah[:], act[:, 0:k])
                    al = chn.tile([P, k], F16, name=nametag + "_al", tag="al",
                                  padded_shape=[P, 32])
                    nc.vector.tensor_tensor(al[:], act[:, 0:k], ah[:], op=ALU.subtract)
                    return ah, al

                def gemm16_cm(act, K, N, wname, gelu):
                    """act: [128, K/128] f32 col-major -> [128, N/128] f32 col-major."""
                    k = K // P
                    ah, al = split16(act, k, wname)
                    rowbuf = chn.tile([1, N], F32, name=wname + "_row", tag="rowbuf",
                                      padded_shape=[1, 4 * DC])
                    nch = (N + 511) // 512
                    for c in range(nch):
                        n0, n1 = c * 512, min(N, (c + 1) * 512)
                        pr = crow.tile([1, 512], F32, name=wname + f"_ps{c}", tag="prow")
                        for kt in range(k):
                            wh = wstr.tile([P, n1 - n0], F16, name=f"{wname}_wh{c}_{kt}",
                                           tag="wh", padded_shape=[P, 512])
                            nc.sync.dma_start(wh[:], wdr_h[wname][kt * P:(kt + 1) * P, n0:n1])
                            wl = wstr.tile([P, n1 - n0], F16, name=f"{wname}_wl{c}_{kt}",
                                           tag="wl", padded_shape=[P, 512])
                            nc.sync.dma_start(wl[:], wdr_l[wname][kt * P:(kt + 1) * P, n0:n1])
                            nc.tensor.matmul(pr[0:1, 0:n1 - n0], ah[:, kt:kt + 1], wh[:],
                                             start=(kt == 0), stop=False)
                            nc.tensor.matmul(pr[0:1, 0:n1 - n0], al[:, kt:kt + 1], wh[:],
                                             start=False, stop=False)
                            nc.tensor.matmul(pr[0:1, 0:n1 - n0], ah[:, kt:kt + 1], wl[:],
                                             start=False, stop=(kt == k - 1))
                        nc.scalar.copy(rowbuf[:, n0:n1], pr[0:1, 0:n1 - n0])
                    out = bounce_to_cm(rowbuf[:], N, wname + "_cm")
                    bt = chn.tile([P, N // P], F32, name=wname + "_b", tag=f"cm{N}b")
                    nc.sync.dma_start(bt[:], bias_cm[wname][:])
                    out2 = chn.tile([P, N // P], F32, name=wname + "_o", tag=f"cm{N}o")
                    nc.vector.tensor_tensor(out2[:], out[:], bt[:], op=ALU.add)
                    if gelu:
                        nc.scalar.activation(out2[:], out2[:], AF.Gelu)
                    return out2

                def ln_stats_cm(act, nfeat):
                    """col-major [128, k] -> (mu, rs) broadcast [P, 2]."""
                    k = nfeat // P
                    ps = cmisc.tile([1, k], F32, name="lnps", tag="lnps", padded_shape=[1, 32])
                    nc.tensor.matmul(ps[:], ones_col[:], act[:, 0:k], start=True, stop=True)
                    srow = chn.tile([1, 1], F32, name="ssum", tag="s11a")
                    nc.vector.tensor_reduce(srow[:], ps[:], axis=AX.X, op=ALU.add)
                    sqt = chn.tile([P, k], F32, name="sqt", tag="sqt", padded_shape=[P, 32])
                    nc.scalar.activation(sqt[:], act[:, 0:k], AF.Square)
                    ps2 = cmisc.tile([1, k], F32, name="lnps2", tag="lnps2", padded_shape=[1, 32])
                    nc.tensor.matmul(ps2[:], ones_col[:], sqt[:], start=True, stop=True)
                    ssq = chn.tile([1, 1], F32, name="ssq", tag="s11b")
                    nc.vector.tensor_reduce(ssq[:], ps2[:], axis=AX.X, op=ALU.add)
                    mu = chn.tile([1, 1], F32, name="lmu", tag="s11c")
                    nc.scalar.mul(mu[:], srow[:], 1.0 / nfeat)
                    msq = chn.tile([1, 1], F32, name="lmsq", tag="s11d")
                    nc.vector.tensor_tensor(msq[:], mu[:], mu[:], op=ALU.mult)
                    var = chn.tile([1, 1], F32, name="lvar", tag="s11e")
                    nc.vector.tensor_scalar(var[:], ssq[:], 1.0 / nfeat, msq[:],
                                            op0=ALU.mult, op1=ALU.subtract)
                    sv = chn.tile([1, 1], F32, name="lsv", tag="s11f")
                    nc.scalar.activation(sv[:], var[:], AF.Sqrt, bias=eps_col[0:1, :])
                    rs = chn.tile([1, 1], F32, name="lrs", tag="s11g")
                    nc.vector.reciprocal(rs[:], sv[:])
                    murs = chn.tile([1, 2], F32, name="lmurs", tag="s12")
                    nc.scalar.copy(murs[:, 0:1], mu[:])
                    nc.scalar.copy(murs[:, 1:2], rs[:])
                    pb = cmisc.tile([P, 2], F32, name="lnbc", tag="lnbc")
                    nc.tensor.matmul(pb[:], ones_row[0:1, 0:P], murs[:], start=True, stop=True)
                    mb = chn.tile([P, 2], F32, name="lmb", tag="s13")
                    nc.scalar.copy(mb[:], pb[:])
                    return mb

                # g finalize: g_cm = gp/S*(pool - sum(rs*mu)) + bp   (col-major)
                g0 = bounce_to_cm(g_row[:], D, "g0")
                gpc = chn.tile([P, D // P], F32, name="gpc", tag="cmgp")
                nc.sync.dma_start(gpc[:], gp_cm[:])
                bpc = chn.tile([P, D // P], F32, name="bpc", tag="cmbp")
                nc.sync.dma_start(bpc[:], bp_cm[:])
                g1t = chn.tile([P, D // P], F32, name="g1t", tag="cmg1")
                nc.vector.tensor_scalar(g1t[:], g0[:], scb[:, 1:2], 1.0 / S,
                                        op0=ALU.subtract, op1=ALU.mult)
                g2t = chn.tile([P, D // P], F32, name="g2t", tag="cmg2")
                nc.vector.tensor_tensor(g2t[:], g1t[:], gpc[:], op=ALU.mult)
                g_cm = chn.tile([P, D // P], F32, name="g_cm", tag="cmg3")
                nc.vector.tensor_tensor(g_cm[:], g2t[:], bpc[:], op=ALU.add)

                # chain (t_embed folded into Wc1x bias host-side)
                x_ctrl = gemm16_cm(g_cm, D, DC, "W_inp", False)
                c1t_ = gemm16_cm(x_ctrl, DC, DC, "Wc1x", True)
                cond = gemm16_cm(c1t_, DC, DC, "Wc2", False)
                gcond = chn.tile([P, DC // P], F32, name="gcond", tag="cmgc")
                nc.scalar.activation(gcond[:], cond[:], AF.Gelu)

                zc = chn.tile([P, DC // P], F32, name="zc0", tag="cmz0")
                nc.sync.dma_start(zc[:], z_cm[:])
                z_cur = zc
                for l in range(L):
                    mod = gemm16_cm(gcond, DC, 4 * DC, f"Wmod{l}", False)  # [128, 32]
                    kk = DC // P
                    s1a, sh1a = mod[:, 0:kk], mod[:, kk:2 * kk]
                    s2a, sh2a = mod[:, 2 * kk:3 * kk], mod[:, 3 * kk:4 * kk]
                    mb = ln_stats_cm(z_cur, DC)
                    lnz = chn.tile([P, kk], F32, name=f"lnz{l}", tag="cmlnz")
                    nc.vector.tensor_scalar(lnz[:], z_cur[:], mb[:, 0:1], mb[:, 1:2],
                                            op0=ALU.subtract, op1=ALU.mult)
                    s1p = chn.tile([P, kk], F32, name=f"s1p{l}", tag="cms1p")
                    nc.scalar.add(s1p[:], s1a, 1.0)
                    h0 = chn.tile([P, kk], F32, name=f"h0_{l}", tag="cmh0")
                    nc.vector.tensor_tensor(h0[:], lnz[:], s1p[:], op=ALU.mult)
                    h1 = chn.tile([P, kk], F32, name=f"h1_{l}", tag="cmh1")
                    nc.vector.tensor_tensor(h1[:], h0[:], sh1a, op=ALU.add)
                    h2 = gemm16_cm(h1, DC, 4 * DC, f"Wm1_{l}", True)
                    h3 = gemm16_cm(h2, 4 * DC, DC, f"Wm2_{l}", False)
                    s2p = chn.tile([P, kk], F32, name=f"s2p{l}", tag="cms2p")
                    nc.scalar.add(s2p[:], s2a, 1.0)
                    h4 = chn.tile([P, kk], F32, name=f"h4_{l}", tag="cmh4")
                    nc.vector.tensor_tensor(h4[:], h3[:], s2p[:], op=ALU.mult)
                    h5 = chn.tile([P, kk], F32, name=f"h5_{l}", tag="cmh5")
                    nc.vector.tensor_tensor(h5[:], h4[:], sh2a, op=ALU.add)
                    zn = chn.tile([P, kk], F32, name=f"zn{l}", tag=f"cmzn{l % 2}")
                    nc.vector.tensor_tensor(zn[:], z_cur[:], h5[:], op=ALU.add)
                    z_cur = zn

                zf = gemm16_cm(z_cur, DC, DC, "Wf", False)
                mb = ln_stats_cm(zf, DC)
                zno = chn.tile([P, DC // P], F32, name="zno", tag="cmzno")
                nc.vector.tensor_scalar(zno[:], zf[:], mb[:, 0:1], mb[:, 1:2],
                                        op0=ALU.subtract, op1=ALU.mult)
                gzc = chn.tile([P, DC // P], F32, name="gzc", tag="cmgz")
                nc.sync.dma_start(gzc[:], gz_cm[:])
                bzc = chn.tile([P, DC // P], F32, name="bzc", tag="cmbz")
                nc.sync.dma_start(bzc[:], bz_cm[:])
                zf2 = chn.tile([P, DC // P], F32, name="zf2", tag="cmzf2")
                nc.vector.tensor_tensor(zf2[:], zno[:], gzc[:], op=ALU.mult)
                z_fin = chn.tile([P, DC // P], F32, name="z_fin", tag="cmzf3")
                nc.vector.tensor_tensor(z_fin[:], zf2[:], bzc[:], op=ALU.add)

                # ---- zW row = z_fin @ W1z  (fp16 3-pass, row-major out) ----
                zfh, zfl = split16(z_fin, DC // P, "zf")
                zw_row = chn.tile([1, DC], F32, name="zw_row", tag="rowzw")
                for c in range(2):
                    pr = crow.tile([1, 512], F32, name=f"zwps{c}", tag="prow")
                    for kt in range(DC // P):
                        wzh = wstr.tile([P, 512], F16, name=f"w1zh_{c}_{kt}", tag="wh")
                        nc.sync.dma_start(wzh[:], W1zh_d[kt * P:(kt + 1) * P, c * 512:(c + 1) * 512])
                        wzl = wstr.tile([P, 512], F16, name=f"w1zl_{c}_{kt}", tag="wl")
                        nc.sync.dma_start(wzl[:], W1zl_d[kt * P:(kt + 1) * P, c * 512:(c + 1) * 512])
                        nc.tensor.matmul(pr[:], zfh[:, kt:kt + 1], wzh[:],
                                         start=(kt == 0), stop=False)
                        nc.tensor.matmul(pr[:], zfl[:, kt:kt + 1], wzh[:],
                                         start=False, stop=False)
                        nc.tensor.matmul(pr[:], zfh[:, kt:kt + 1], wzl[:],
                                         start=False, stop=(kt == DC // P - 1))
                    nc.scalar.copy(zw_row[:, c * 512:(c + 1) * 512], pr[:])

                # ---- z scalar stats for router LN ----
                zsq = chn.tile([P, DC // P], F32, name="zsq", tag="cmzsq")
                nc.scalar.activation(zsq[:], z_fin[:], AF.Square)
                psa = cmisc.tile([1, DC // P], F32, name="zsps", tag="lnps", padded_shape=[1, 32])
                nc.tensor.matmul(psa[:], ones_col[:], z_fin[:], start=True, stop=True)
                psb = cmisc.tile([1, DC // P], F32, name="zsps2", tag="lnps2", padded_shape=[1, 32])
                nc.tensor.matmul(psb[:], ones_col[:], zsq[:], start=True, stop=True)
                zsr = chn.tile([1, 2], F32, name="zsr", tag="s12b")
                nc.vector.tensor_reduce(zsr[:, 0:1], psa[:], axis=AX.X, op=ALU.add)
                nc.vector.tensor_reduce(zsr[:, 1:2], psb[:], axis=AX.X, op=ALU.add)
                pzb = cmisc.tile([P, 2], F32, name="pzb", tag="lnbc")
                nc.tensor.matmul(pzb[:], ones_row[0:1, 0:P], zsr[:], start=True, stop=True)
                zsb = const.tile([P, 2], F32)
                nc.scalar.copy(zsb[:], pzb[:])

                # ---- router per-token stats [128, NT] ----
                DD = D + DC
                mur = const.tile([P, NT], F32)
                nc.vector.tensor_scalar(mur[:], rsum_t[:].broadcast_to([P, NT]), zsb[:, 0:1],
                                        1.0 / DD, op0=ALU.add, op1=ALU.mult)
                mq2 = chn.tile([P, NT], F32, name="mq2", tag="st1")
                nc.vector.tensor_tensor(mq2[:], mur[:], mur[:], op=ALU.mult)
                vr = chn.tile([P, NT], F32, name="vr", tag="st2")
                nc.vector.tensor_scalar(vr[:], rssq_t[:], zsb[:, 1:2], 1.0 / DD,
                                        op0=ALU.add, op1=ALU.mult)
                vr2 = chn.tile([P, NT], F32, name="vr2", tag="st3")
                nc.vector.tensor_tensor(vr2[:], vr[:], mq2[:], op=ALU.subtract)
                irs = const.tile([P, NT], F32)  # 1/rs = sqrt(var+eps)
                nc.scalar.activation(irs[:], vr2[:], AF.Sqrt, bias=eps_col[:])
                rst = const.tile([P, NT], F32)
                nc.vector.reciprocal(rst[:], irs[:])

                # fp16 h/l splits of mur, irs
                murh = const.tile([P, NT], F16)
                nc.scalar.copy(murh[:], mur[:])
                murl = const.tile([P, NT], F16)
                nc.vector.tensor_tensor(murl[:], mur[:], murh[:], op=ALU.subtract)
                irsh = const.tile([P, NT], F16)
                nc.scalar.copy(irsh[:], irs[:])
                irsl = const.tile([P, NT], F16)
                nc.vector.tensor_tensor(irsl[:], irs[:], irsh[:], op=ALU.subtract)

                # corr_mov8 rows: [mu_h, irs_h, 1, mu_h, irs_h, 1, mu_l, irs_l]
                ones16 = chn.tile([1, S], F16, name="ones16", tag="ones16")
                nc.vector.memset(ones16[:], 1.0)
                o16scr = dramp.tile([S], F16, name="o16scr", tag="o16scr")
                nc.sync.dma_start(o16scr[None, :], ones16[:])
                nc.sync.dma_start(corr_mov8[2:3, :], o16scr[None, :])
                nc.sync.dma_start(corr_mov8[5:6, :], o16scr[None, :])
                for srcst, rows, tg in ((murh, (0, 3), "a"), (irsh, (1, 4), "b"),
                                        (murl, (6,), "c"), (irsl, (7,), "d")):
                    scr = dramp.tile([P, NT], F16, name=f"st16{tg}", tag="stscr16")
                    nc.sync.dma_start(scr[:], srcst[:])
                    for r in rows:
                        nc.sync.dma_start(
                            corr_mov8[r:r + 1, :].rearrange("o (t p) -> o t p", t=NT),
                            scr.rearrange("p t -> t p"))
                # rs_row fp32 (for the pre-gelu scale)
                scr = dramp.tile([P, NT], F32, name="stscr2", tag="stscr")
                nc.sync.dma_start(scr[:], rst[:])
                nc.sync.dma_start(rs_row[:].rearrange("o (t p) -> o t p", t=NT),
                                  scr.rearrange("p t -> t p"))

                # corr_lhsT8 rows: [-s1_h, c1_h, zW_h, -s1_l, c1_l, zW_l, -s1_h, c1_h]
                nc.sync.dma_start(corr_lhsT8[0:1, :], corr_sc_h[0:1, :])
                nc.sync.dma_start(corr_lhsT8[6:7, :], corr_sc_h[0:1, :])
                nc.sync.dma_start(corr_lhsT8[1:2, :], corr_sc_h[1:2, :])
                nc.sync.dma_start(corr_lhsT8[7:8, :], corr_sc_h[1:2, :])
                nc.sync.dma_start(corr_lhsT8[3:4, :], corr_sc_l[0:1, :])
                nc.sync.dma_start(corr_lhsT8[4:5, :], corr_sc_l[1:2, :])
                zwh = chn.tile([1, DC], F16, name="zwh", tag="rowzwh")
                nc.scalar.copy(zwh[:], zw_row[:])
                zwl = chn.tile([1, DC], F16, name="zwl", tag="rowzwl")
                nc.vector.tensor_tensor(zwl[:], zw_row[:], zwh[:], op=ALU.subtract)
                zwscrh = dramp.tile([DC], F16, name="zwscrh", tag="zwscr16h")
                nc.sync.dma_start(zwscrh[None, :], zwh[:])
                nc.sync.dma_start(corr_lhsT8[2:3, :], zwscrh[None, :])
                zwscrl = dramp.tile([DC], F16, name="zwscrl", tag="zwscr16l")
                nc.sync.dma_start(zwscrl[None, :], zwl[:])
                nc.sync.dma_start(corr_lhsT8[5:6, :], zwscrl[None, :])

        # =========== PASS 2: router (fp16 split GEMMs) ===========
        w1xh_sb = const.tile([P, D // P, DC], F16)
        w1xl_sb = const.tile([P, D // P, DC], F16)
        for kt in range(D // P):
            nc.sync.dma_start(w1xh_sb[:, kt, :], W1xh_d[kt * P:(kt + 1) * P, :])
            if g1_passes == 3:
                nc.sync.dma_start(w1xl_sb[:, kt, :], W1xl_d[kt * P:(kt + 1) * P, :])
        wr2h_sb = const.tile([P, DC // P, H], F16)
        wr2l_sb = const.tile([P, DC // P, H], F16)
        for kt in range(DC // P):
            nc.sync.dma_start(wr2h_sb[:, kt, :], Wr2h_d[kt * P:(kt + 1) * P, :])
            nc.sync.dma_start(wr2l_sb[:, kt, :], Wr2l_d[kt * P:(kt + 1) * P, :])
        br2_sb = const.tile([1, H], F32)
        nc.sync.dma_start(br2_sb[:], br2d[:])

        with tc.tile_pool(name="p2x", bufs=3) as p2x, \
             tc.tile_pool(name="p2xt", bufs=2) as p2xt, \
             tc.tile_pool(name="p2g", bufs=2) as p2g, \
             tc.tile_pool(name="p2s", bufs=3) as p2s, \
             tc.tile_pool(name="pT", bufs=2, space="PSUM") as pT, \
             tc.tile_pool(name="pG", bufs=2, space="PSUM") as pG, \
             tc.tile_pool(name="pM", bufs=1, space="PSUM") as pM:
            for g in range(NG):
                xtgh = p2xt.tile([P, D // P, GRP], F16, name=f"xtgh{g}", tag="xtgh")
                xtgl = p2xt.tile([P, D // P, GRP], F16, name=f"xtgl{g}", tag="xtgl")
                for j in range(TPG):
                    i = g * TPG + j
                    x = p2x.tile([P, D], F32, name=f"x2_{i}", tag="x2")
                    nc.sync.dma_start(x[:], Xd[i * P:(i + 1) * P, :])
                    for bblk in range(D // P):
                        pt = pT.tile([P, P], F32, name=f"pt{i}_{bblk}", tag="pt")
                        nc.tensor.transpose(pt[:], x[:, bblk * P:(bblk + 1) * P], ident[:])
                        nc.scalar.copy(xtgh[:, bblk, j * P:(j + 1) * P], pt[:])
                        nc.vector.tensor_tensor(xtgl[:, bblk, j * P:(j + 1) * P], pt[:],
                                                xtgh[:, bblk, j * P:(j + 1) * P],
                                                op=ALU.subtract)
                # rs broadcast [128, GRP]
                pb = pM.tile([P, GRP], F32, name=f"pb{g}", tag="pbg")
                nc.tensor.matmul(pb[:], ones_row[0:1, 0:P],
                                 rs_row[0:1, g * GRP:(g + 1) * GRP], start=True, stop=True)
                rsb = p2s.tile([P, GRP], F32, name=f"rsb{g}", tag="rsb")
                nc.vector.tensor_copy(rsb[:], pb[:])

                g1h = p2g.tile([P, DC // P, GRP], F16, name=f"g1h_{g}", tag="g1h")
                g1l = p2g.tile([P, DC // P, GRP], F16, name=f"g1l_{g}", tag="g1l")
                for n in range(DC // P):
                    pg = pG.tile([P, GRP], F32, name=f"pg{g}_{n}", tag="pg")
                    for kt in range(D // P):
                        nc.tensor.matmul(pg[:], w1xh_sb[:, kt, n * P:(n + 1) * P],
                                         xtgh[:, kt, :], start=(kt == 0), stop=False)
                        nc.tensor.matmul(pg[:], w1xh_sb[:, kt, n * P:(n + 1) * P],
                                         xtgl[:, kt, :], start=False, stop=False)
                        if g1_passes == 3:
                            nc.tensor.matmul(pg[:], w1xl_sb[:, kt, n * P:(n + 1) * P],
                                             xtgh[:, kt, :], start=False, stop=False)
                    nc.tensor.matmul(pg[:], corr_lhsT8[:, n * P:(n + 1) * P],
                                     corr_mov8[:, g * GRP:(g + 1) * GRP],
                                     start=False, stop=True)
                    pre = p2s.tile([P, GRP], F32, name=f"pre{g}_{n}", tag="pre")
                    nc.vector.tensor_tensor(pre[:], pg[:], rsb[:], op=ALU.mult)
                    g32 = p2s.tile([P, GRP], F32, name=f"g32_{g}_{n}", tag="g32")
                    nc.scalar.activation(g32[:], pre[:], AF.Gelu)
                    nc.vector.tensor_copy(g1h[:, n, :], g32[:])
                    nc.vector.tensor_tensor(g1l[:, n, :], g32[:], g1h[:, n, :],
                                            op=ALU.subtract)

                # GEMM2 -> logits2^T [16, GRP] (fp16 3-pass)
                pl = pM.tile([H, GRP], F32, name=f"pl{g}", tag="pl")
                for kt in range(DC // P):
                    nc.tensor.matmul(pl[:], wr2h_sb[:, kt, :], g1h[:, kt, :],
                                     start=(kt == 0), stop=False)
                    nc.tensor.matmul(pl[:], wr2h_sb[:, kt, :], g1l[:, kt, :],
                                     start=False, stop=False)
                    nc.tensor.matmul(pl[:], wr2l_sb[:, kt, :], g1h[:, kt, :],
                                     start=False, stop=False)
                nc.tensor.matmul(pl[:], br2_sb[:], ones_row[0:1, 0:GRP],
                                 start=False, stop=True)
                l2t = p2s.tile([H, GRP], F32, name=f"l2t{g}", tag="l2t")
                nc.scalar.copy(l2t[:], pl[:])

                for j in range(TPG):
                    ptb = pM.tile([P, H], F32, name=f"ptb{g}_{j}", tag="ptb")
                    nc.tensor.transpose(ptb[:], l2t[:, j * P:(j + 1) * P],
                                        ident[0:H, 0:H])
                    e = p2s.tile([P, H], F32, name=f"e{g}{j}", tag="te")
                    nc.scalar.activation(e[:], ptb[:], AF.Exp)
                    m1 = p2s.tile([P, 1], F32, name=f"m1{g}{j}", tag="tm1")
                    nc.vector.reduce_max(m1[:], e[:], axis=AX.X)
                    mask = p2s.tile([P, H], F32, name=f"mk{g}{j}", tag="tmk")
                    nc.vector.tensor_scalar(mask[:], e[:], m1[:], None, op0=ALU.is_ge)
                    e2 = p2s.tile([P, H], F32, name=f"e2{g}{j}", tag="te2")
                    nc.vector.scalar_tensor_tensor(e2[:], in0=mask[:], scalar=-1e30,
                                                   in1=e[:], op0=ALU.mult, op1=ALU.add)
                    m2 = p2s.tile([P, 1], F32, name=f"m2{g}{j}", tag="tm2")
                    nc.vector.reduce_max(m2[:], e2[:], axis=AX.X)
                    den = p2s.tile([P, 1], F32, name=f"dn{g}{j}", tag="tdn")
                    nc.vector.tensor_tensor(den[:], m1[:], m2[:], op=ALU.add)
                    rden = p2s.tile([P, 1], F32, name=f"rd{g}{j}", tag="trd")
                    nc.vector.reciprocal(rden[:], den[:])
                    keep = p2s.tile([P, H], F32, name=f"kp{g}{j}", tag="tkp")
                    nc.vector.tensor_scalar(keep[:], e[:], m2[:], None, op0=ALU.is_ge)
                    alph = p2s.tile([P, H], F32, name=f"al{g}{j}", tag="tal")
                    nc.vector.scalar_tensor_tensor(alph[:], in0=e[:], scalar=rden[:],
                                                   in1=keep[:], op0=ALU.mult, op1=ALU.mult)
                    i = g * TPG + j
                    nc.sync.dma_start(alphad[i * P:(i + 1) * P, :], alph[:])

    nc.compile()
    return nc


def _cm(v):
    v = np.asarray(v, np.float32).reshape(-1)
    return np.ascontiguousarray(v.reshape(-1, P).T)


def _f16_split(W):
    import ml_dtypes
    bf16 = ml_dtypes.bfloat16
    W = np.asarray(W, np.float64)
    Wh = np.asarray(W, np.float32).astype(bf16)
    Wl = np.asarray(W - Wh.astype(np.float64), np.float32).astype(bf16)
    return np.ascontiguousarray(Wh), np.ascontiguousarray(Wl)


def _host_prep(inputs):
    X = np.asarray(inputs['X'], np.float32)
    z = np.asarray(inputs['z'], np.float32)
    step_idx = int(inputs['step_idx'])
    t = np.linspace(0.0, 1.0, MAX_DEPTH)
    sig = float(np.clip(np.cos(t * (math.pi / 2)), 1e-4, None)[min(step_idx, MAX_DEPTH - 1)])
    half = DC // 2
    freqs = np.exp(-math.log(10000.0) * np.arange(half, dtype=np.float64) / half)
    args = sig * freqs
    emb = np.concatenate([np.cos(args), np.sin(args)])  # [DC] fp64

    # host-computed t_embed (depends only on step_idx)
    from scipy.special import erf

    def gelu64(x):
        return 0.5 * x * (1 + erf(x / np.sqrt(2.0)))

    Wt1 = np.asarray(inputs['Wt1'], np.float64)
    bt1 = np.asarray(inputs['bt1'], np.float64)
    Wt2 = np.asarray(inputs['Wt2'], np.float64)
    bt2 = np.asarray(inputs['bt2'], np.float64)
    t_embed = gelu64(emb @ Wt1 + bt1) @ Wt2 + bt2  # [DC]

    Wc1 = np.asarray(inputs['Wc1'], np.float64)
    bc1 = np.asarray(inputs['bc1'], np.float64)
    cond_bias = t_embed @ Wc1[DC:] + bc1           # [DC]
    Wc1x = Wc1[:DC]

    gr = np.asarray(inputs['gr'], np.float64)
    br = np.asarray(inputs['br'], np.float64)
    Wr1 = np.asarray(inputs['Wr1'], np.float64)
    W1p = gr[:, None] * Wr1
    W1x = W1p[:D]
    W1z = W1p[D:]
    s1 = W1p.sum(0)
    br1 = np.asarray(inputs['br1'], np.float64)
    c1 = br @ Wr1 + br1

    W1xh, W1xl = _f16_split(W1x)
    W1zh, W1zl = _f16_split(W1z)
    Wr2h, Wr2l = _f16_split(inputs['Wr2'])
    import ml_dtypes
    bf16 = ml_dtypes.bfloat16
    csc = np.stack([-s1, c1])
    csc_h = np.asarray(csc, np.float32).astype(bf16)
    csc_l = np.asarray(csc - csc_h.astype(np.float64), np.float32).astype(bf16)

    shared = {
        'gp_cm': _cm(inputs['g_pool']), 'bp_cm': _cm(inputs['b_pool']),
        'gz_cm': _cm(inputs['gz']), 'bz_cm': _cm(inputs['bz']),
        'W1xh': W1xh, 'W1xl': W1xl, 'W1zh': W1zh, 'W1zl': W1zl,
        'corr_sc_h': np.ascontiguousarray(csc_h),
        'corr_sc_l': np.ascontiguousarray(csc_l),
        'Wr2h': Wr2h, 'Wr2l': Wr2l,
        'ones_s': np.ones((1, S), np.float32),
        'br2': np.asarray(inputs['br2'], np.float32)[None, :],
    }
    cw = {'W_inp': inputs['W_inp'], 'Wc1x': Wc1x, 'Wc2': inputs['Wc2'],
          'Wf': inputs['Wf']}
    cb = {'W_inp': inputs['b_inp'], 'Wc1x': cond_bias, 'Wc2': inputs['bc2'],
          'Wf': inputs['bf']}
    for l in range(L):
        cw[f'Wmod{l}'] = np.asarray(inputs['W_mod'])[l]
        cb[f'Wmod{l}'] = np.asarray(inputs['b_mod'])[l]
        cw[f'Wm1_{l}'] = np.asarray(inputs['Wm1'])[l]
        cb[f'Wm1_{l}'] = np.asarray(inputs['bm1'])[l]
        cw[f'Wm2_{l}'] = np.asarray(inputs['Wm2'])[l]
        cb[f'Wm2_{l}'] = np.asarray(inputs['bm2'])[l]
    for k, v in cw.items():
        h, lo = _f16_split(v)
        shared[k + '_h'] = h
        shared[k + '_l'] = lo
        shared['b_' + k] = _cm(np.asarray(cb[k], np.float32))

    in_maps = []
    for c in range(B):
        m = dict(shared)
        m['X'] = np.ascontiguousarray(X[c])
        m['z_cm'] = _cm(z[c])
        in_maps.append(m)
    return in_maps


def get_nc():
    key = ('nc', G1_PASSES)
    if key not in _CACHE:
        _CACHE[key] = _build(g1_passes=G1_PASSES)
    return _CACHE[key]


def kernel(**inputs):
    from concourse.bass_utils import run_bass_kernel_spmd
    nc = get_nc()
    in_maps = _host_prep(inputs)
    res = run_bass_kernel_spmd(nc, in_maps, list(range(B)))
    out = np.stack([res.results[c]['alpha'] for c in range(B)], axis=0)
    return out.astype(np.float32)
